# revision 14
# baseline (speedup 1.0000x reference)
"""Guided filter (radius=3) on 8x TRN2 NeuronCores, batch-parallel.

Per core: one image. Box filters = banded matmuls on the PE:
  a1: lhsT = image block (stationary), rhs = vertical band -> (w, h') transposed
  a2: lhsT = horizontal band (stationary), rhs = a1 evac    -> (w', h')
Stage-2 (boxes of a, b) repeats the pair, returning to natural layout.

v5: phase-2(c) is interleaved with stage-1(c+1) so the PE-heavy stage-2
windows overlap the Vector/Scalar-heavy stage-1 windows. ab tiles are
double-buffered (bufs=2) so channel c+1's coefficient writes never wait on
channel c's stage-2 reads. I*p product strips are computed one step ahead
on GpSimd (c0 on Vector). ifull is reloaded per channel through a 2-buf
rotation; c0-only tiles share tags with phase-2-only tiles to fit SBUF.
"""

import sys

sys.path.insert(0, "/opt/trn_rl_repo")

import numpy as np
import ml_dtypes

R = 3
H = W = 1024
P = 128
NC_N = 8
V = 122  # valid outputs per 128-wide band matmul
S = float(64.0 / 49.0)

_cache = {}


def _strips():
    # (in_lo, in_hi, out_lo, out_hi) along one axis
    out = []
    j = 0
    while j * V < W:
        o_lo, o_hi = j * V, min(W, j * V + V)
        i_lo, i_hi = max(0, o_lo - R), min(W, o_hi + R)
        out.append((i_lo, i_hi, o_lo, o_hi))
        j += 1
    return out


def _band7_np():
    b = np.zeros((128, 134), np.float32)
    for k in range(128):
        for d in range(134):
            if abs(d - 3 - k) <= R:
                b[k, d] = 0.125
    return b.astype(ml_dtypes.bfloat16)


def _bandm_np(i_lo, i_hi, o_lo, o_hi):
    K = i_hi - i_lo
    bm = np.zeros((K, 128), np.float32)
    for k in range(K):
        for m in range(o_hi - o_lo):
            if abs((i_lo + k) - (o_lo + m)) <= R:
                bm[k, m] = 0.125
    return bm.astype(ml_dtypes.bfloat16)


def _seg512(lo, hi):
    """split [lo,hi) at multiples of 512 (PSUM bank boundaries)"""
    segs = []
    while lo < hi:
        nxt = min(hi, (lo // 512 + 1) * 512)
        segs.append((lo, nxt))
        lo = nxt
    return segs


def _build():
    import concourse.bass as bass
    import concourse.bacc as bacc
    import concourse.mybir as mybir
    from concourse import tile

    bf16 = mybir.dt.bfloat16
    f32 = mybir.dt.float32
    Copy = mybir.ActivationFunctionType.Copy
    Alu = mybir.AluOpType

    strips = _strips()
    NS = len(strips)

    nc = bacc.Bacc(None, target_bir_lowering=False)
    dI = nc.dram_tensor("I", [H, W], f32, kind="ExternalInput")
    dp = nc.dram_tensor("p", [3, H, W], f32, kind="ExternalInput")
    db7 = nc.dram_tensor("band7", [128, 134], bf16, kind="ExternalInput")
    dbm_f = nc.dram_tensor("bandm_first", [125, 128], bf16, kind="ExternalInput")
    dbm_i = nc.dram_tensor("bandm_int", [128, 128], bf16, kind="ExternalInput")
    dbm_l = nc.dram_tensor("bandm_last", [51, 128], bf16, kind="ExternalInput")
    dq = nc.dram_tensor("q", [3, H, W], f32, kind="ExternalOutput")

    with tile.TileContext(nc) as tc:
        with (
            tc.tile_pool(name="const", bufs=1) as constp,
            tc.tile_pool(name="chk", bufs=1) as chp,
            tc.tile_pool(name="perm", bufs=1) as perm,
            tc.tile_pool(name="work", bufs=2) as workp,
            tc.tile_pool(name="ab", bufs=2) as abp,
            tc.tile_pool(name="qo", bufs=2) as qp,
            tc.tile_pool(name="psA", bufs=3, space="PSUM") as psA,
        ):
            band7 = constp.tile([128, 134], bf16, tag="band7")
            nc.sync.dma_start(band7[:], db7.ap()[:])
            bm_first = constp.tile([125, 128], bf16, tag="bmf")
            nc.sync.dma_start(bm_first[:], dbm_f.ap()[:])
            bm_int = constp.tile([128, 128], bf16, tag="bmi")
            nc.sync.dma_start(bm_int[:], dbm_i.ap()[:])
            bm_last = constp.tile([51, 128], bf16, tag="bml")
            nc.sync.dma_start(bm_last[:], dbm_l.ap()[:])

            def bandm_for(si):
                if si == 0:
                    return bm_first
                if si == NS - 1:
                    return bm_last
                return bm_int

            # ---- persistent natural-layout input chunks (gpsimd cast-DMA)
            Ich = []
            for k in range(8):
                Ick = chp.tile([128, 1024], bf16, tag=f"I{k}")
                nc.gpsimd.dma_start(Ick[:], dI.ap()[128 * k : 128 * k + 128, :])
                Ich.append(Ick)

            pch = {}

            def load_p_chunk(c, k):
                pck = chp.tile([128, 1024], bf16, tag=f"p{k}", bufs=2)
                nc.gpsimd.dma_start(pck[:], dp.ap()[c][128 * k : 128 * k + 128, :])
                pch.setdefault(c, {})[k] = pck

            ipstrip = {}

            def ip_strip(c, si, eng):
                """packed I*p product strip [128, 8*128] (block i at i*128)"""
                i_lo, i_hi, _, _ = strips[si]
                Mw = i_hi - i_lo
                t = workp.tile([128, 8 * 128], bf16, tag="ipps", bufs=2)
                for i in range(8):
                    eng.tensor_mul(
                        t[:, i * 128 : i * 128 + Mw],
                        Ich[i][:, i_lo:i_hi],
                        pch[c][i][:, i_lo:i_hi],
                    )
                ipstrip[(c, si)] = t

            def a1_pass(ps, blocks, si, Mw, stride=None):
                """vertical box + transpose: accumulate 8 h-blocks into
                ps[0:Mw, 0:1024]."""
                i_lo, i_hi, _, _ = strips[si]
                seen = set()
                for i in range(8):
                    if stride is None:
                        lhsT = blocks[i][:, i_lo:i_hi]
                    else:
                        lhsT = blocks[:, i * stride : i * stride + Mw]
                    base = 128 * i - 3
                    w_lo_ = max(0, 128 * i - 3)
                    w_hi_ = min(1024, 128 * i + 131)
                    for s_lo, s_hi in _seg512(w_lo_, w_hi_):
                        bank = s_lo // 512
                        nc.tensor.matmul(
                            ps[0:Mw, s_lo:s_hi],
                            lhsT,
                            band7[:, s_lo - base : s_hi - base],
                            start=bank not in seen,
                            stop=True,
                        )
                        seen.add(bank)

            def a2_pass(ps, vt_tile, si, K):
                """horizontal box via band-stationary matmul: ps[0:128, 0:1024]"""
                bm = bandm_for(si)
                for s_lo, s_hi in _seg512(0, 1024):
                    nc.tensor.matmul(
                        ps[:, s_lo:s_hi],
                        bm[:],
                        vt_tile[0:K, s_lo:s_hi],
                        start=True,
                        stop=True,
                    )

            uI_t = {}
            rv_t = {}
            A_t = {}
            B_t = {}

            def b1_pass(ps, tiles, m_lo, m_hi):
                """W-box of a/b over w'-strips + transpose back to natural"""
                seen = set()
                for sj, (ji_lo, ji_hi, jo_lo, jo_hi) in enumerate(strips):
                    Kj = jo_hi - jo_lo
                    t = tiles[sj]
                    lhsT = t[0:Kj, m_lo:m_hi]
                    base = jo_lo - 3
                    w_lo_ = max(0, jo_lo - 3)
                    w_hi_ = min(1024, jo_lo + 125)
                    for s_lo, s_hi in _seg512(w_lo_, w_hi_):
                        bank = s_lo // 512
                        nc.tensor.matmul(
                            ps[0 : m_hi - m_lo, s_lo:s_hi],
                            lhsT,
                            band7[0:Kj, s_lo - base : s_hi - base],
                            start=bank not in seen,
                            stop=True,
                        )
                        seen.add(bank)

            def s1_strip(c, si):
                """stage-1 for one strip: PE a1s then a2s; ACT evacs; DVE math."""
                i_lo, i_hi, o_lo, o_hi = strips[si]
                Mw = i_hi - i_lo
                K_out = o_hi - o_lo
                pc = pch[c]
                if c == 0:
                    # ii strip (packed, DVE) and ip strip (DVE, inline)
                    iis = workp.tile([128, 8 * 128], bf16, tag="iis", bufs=1)
                    ii_eng = nc.vector if si < 1 else nc.gpsimd
                    for i in range(8):
                        ii_eng.tensor_mul(
                            iis[:, i * 128 : i * 128 + Mw],
                            Ich[i][:, i_lo:i_hi],
                            Ich[i][:, i_lo:i_hi],
                        )
                    ip_strip(0, si, nc.vector)

                    psa = psA.tile([128, 1024], f32, tag="mm")
                    a1_pass(psa, Ich, si, Mw)
                    vtI = workp.tile([128, 1024], bf16, tag="vt", bufs=3)
                    nc.scalar.activation(vtI[0:Mw, :], psa[0:Mw, :], Copy, bias=0.0, scale=1.0)

                    psa2 = psA.tile([128, 1024], f32, tag="mm")
                    a1_pass(psa2, iis, si, Mw, stride=128)
                    vtII = workp.tile([128, 1024], bf16, tag="vt", bufs=3)
                    nc.scalar.activation(vtII[0:Mw, :], psa2[0:Mw, :], Copy, bias=0.0, scale=1.0)

                    psa3 = psA.tile([128, 1024], f32, tag="mm")
                    a1_pass(psa3, pc, si, Mw)
                    vtp = workp.tile([128, 1024], bf16, tag="vt", bufs=3)
                    nc.scalar.activation(vtp[0:Mw, :], psa3[0:Mw, :], Copy, bias=0.0, scale=1.0)

                    psa4 = psA.tile([128, 1024], f32, tag="mm")
                    a1_pass(psa4, ipstrip[(c, si)], si, Mw, stride=128)
                    vtip = workp.tile([128, 1024], bf16, tag="vt", bufs=3)
                    nc.scalar.activation(vtip[0:Mw, :], psa4[0:Mw, :], Copy, bias=0.0, scale=1.0)

                    psb = psA.tile([128, 1024], f32, tag="mm")
                    a2_pass(psb, vtI, si, Mw)
                    uI = perm.tile([128, 1024], bf16, tag=f"uI{si}")
                    nc.scalar.activation(uI[0:K_out, :], psb[0:K_out, :], Copy, bias=0.0, scale=S)
                    uI_t[si] = uI

                    psb2 = psA.tile([128, 1024], f32, tag="hold", bufs=1)
                    a2_pass(psb2, vtII, si, Mw)  # held in PSUM; var reads it

                    psb3 = psA.tile([128, 1024], f32, tag="mm")
                    a2_pass(psb3, vtp, si, Mw)
                    up = workp.tile([128, 1024], bf16, tag="up", bufs=2)
                    nc.scalar.activation(up[0:K_out, :], psb3[0:K_out, :], Copy, bias=0.0, scale=S)

                    psb4 = psA.tile([128, 1024], f32, tag="mm")
                    a2_pass(psb4, vtip, si, Mw)
                    uip = workp.tile([128, 1024], bf16, tag="uip", bufs=2)
                    nc.scalar.activation(uip[0:K_out, :], psb4[0:K_out, :], Copy, bias=0.0, scale=S)

                    # var chain: sq (TT 2x, shares t3 slot), var = S*uII - sq, recip
                    sq = workp.tile([128, 1024], bf16, tag="t3", bufs=1)
                    nc.vector.tensor_mul(sq[0:K_out, :], uI[0:K_out, :], uI[0:K_out, :])
                    var = workp.tile([128, 1024], f32, tag="var", bufs=1)
                    nc.vector.scalar_tensor_tensor(
                        var[0:K_out, :], psb2[0:K_out, :], S, sq[0:K_out, :], Alu.mult, Alu.subtract
                    )
                    rvf = workp.tile([128, 1024], f32, tag="rvf", bufs=1)
                    nc.vector.reciprocal_approx_fast(rvf[0:K_out, :], var[0:K_out, :])
                    rv = perm.tile([128, 1024], bf16, tag=f"rv{si}")
                    nc.scalar.activation(rv[0:K_out, :], rvf[0:K_out, :], Copy, bias=0.0, scale=1.0)
                    rv_t[si] = rv
                else:
                    psa3 = psA.tile([128, 1024], f32, tag="mm")
                    a1_pass(psa3, pc, si, Mw)
                    vtp = workp.tile([128, 1024], bf16, tag="vt", bufs=3)
                    nc.scalar.activation(vtp[0:Mw, :], psa3[0:Mw, :], Copy, bias=0.0, scale=1.0)

                    psa4 = psA.tile([128, 1024], f32, tag="mm")
                    a1_pass(psa4, ipstrip[(c, si)], si, Mw, stride=128)
                    vtip = workp.tile([128, 1024], bf16, tag="vt", bufs=3)
                    nc.scalar.activation(vtip[0:Mw, :], psa4[0:Mw, :], Copy, bias=0.0, scale=1.0)

                    psb3 = psA.tile([128, 1024], f32, tag="mm")
                    a2_pass(psb3, vtp, si, Mw)
                    up = workp.tile([128, 1024], bf16, tag="up", bufs=2)
                    nc.scalar.activation(up[0:K_out, :], psb3[0:K_out, :], Copy, bias=0.0, scale=S)

                    psb4 = psA.tile([128, 1024], f32, tag="hold", bufs=1)
                    a2_pass(psb4, vtip, si, Mw)
                    uip = None

                uI = uI_t[si]
                rv = rv_t[si]
                tt = workp.tile([128, 1024], bf16, tag="tt", bufs=1)
                nc.vector.tensor_mul(tt[0:K_out, :], uI[0:K_out, :], up[0:K_out, :])
                cov = workp.tile([128, 1024], bf16, tag="cov", bufs=1)
                if uip is not None:
                    nc.vector.tensor_sub(cov[0:K_out, :], uip[0:K_out, :], tt[0:K_out, :])
                else:
                    nc.vector.scalar_tensor_tensor(
                        cov[0:K_out, :], psb4[0:K_out, :], S, tt[0:K_out, :], Alu.mult, Alu.subtract
                    )
                a_t = abp.tile([128, 1024], bf16, tag=f"a{si}")
                nc.vector.tensor_mul(a_t[0:K_out, :], cov[0:K_out, :], rv[0:K_out, :])
                t3 = workp.tile([128, 1024], bf16, tag="t3", bufs=1)
                nc.vector.tensor_mul(t3[0:K_out, :], a_t[0:K_out, :], uI[0:K_out, :])
                b_t = abp.tile([128, 1024], bf16, tag=f"b{si}")
                nc.vector.tensor_sub(b_t[0:K_out, :], up[0:K_out, :], t3[0:K_out, :])
                A_t[si] = a_t
                B_t[si] = b_t

            ifull = {}

            def load_ifull(c, m):
                if m >= NS:
                    return
                _, _, mo_lo, mo_hi = strips[m]
                ift = perm.tile([128, 1024], bf16, tag="if", bufs=2)
                r = mo_lo
                while r < mo_hi:
                    k = r // 128
                    r_hi = min(mo_hi, 128 * (k + 1))
                    nc.sync.dma_start(
                        ift[r - mo_lo : r_hi - mo_lo, :],
                        Ich[k][r - 128 * k : r_hi - 128 * k, :],
                    )
                    r = r_hi
                ifull[(c, m)] = ift

            def do_finals(c, mm_i, abox_t, bbox_t, Aview, Bview):
                m2i_lo, m2i_hi, m2o_lo, m2o_hi = strips[mm_i]
                Hw = m2o_hi - m2o_lo
                Kf = m2i_hi - m2i_lo
                bm2 = bandm_for(mm_i)
                qbf = qp.tile([128, 1024], bf16, tag="qb", bufs=2)
                for s_lo, s_hi in _seg512(0, 1024):
                    pd = psA.tile([128, 1024], f32, tag="mm")
                    nc.tensor.matmul(
                        pd[:, 0:512], bm2[:], abox_t[0:Kf, s_lo:s_hi], start=True, stop=True
                    )
                    nc.tensor.matmul(
                        pd[:, 512:1024], bm2[:], bbox_t[0:Kf, s_lo:s_hi], start=True, stop=True
                    )
                    tseg = workp.tile([128, 512], bf16, tag="iis", bufs=1)
                    nc.vector.tensor_mul(
                        tseg[0:Hw, :], ifull[(c, mm_i)][0:Hw, s_lo:s_hi], pd[0:Hw, 0:512]
                    )
                    nc.vector.tensor_add(
                        qbf[0:Hw, s_lo:s_hi], tseg[0:Hw, :], pd[0:Hw, 512:1024]
                    )
                nc.vector.tensor_scalar(qbf[0:Hw, :], qbf[0:Hw, :], 1.0, 0.0, Alu.min, Alu.max)
                nc.gpsimd.dma_start(dq.ap()[c][m2o_lo:m2o_hi, :], qbf[0:Hw, :])

            def ph2_m(c, m, Acur, Bcur, pend):
                """phase-2 step: b1 passes + evacs for m, finals for pend"""
                mi_lo, mi_hi, mo_lo, mo_hi = strips[m]
                K2 = mi_hi - mi_lo
                psc_a = psA.tile([128, 1024], f32, tag="mm")
                b1_pass(psc_a, Acur, mi_lo, mi_hi)
                abox = workp.tile([128, 1024], bf16, tag="abox", bufs=2)
                nc.scalar.activation(abox[0:K2, :], psc_a[0:K2, :], Copy, bias=0.0, scale=S)
                psc_b = psA.tile([128, 1024], f32, tag="mm")
                b1_pass(psc_b, Bcur, mi_lo, mi_hi)
                bbox = workp.tile([128, 1024], bf16, tag="bbox", bufs=2)
                nc.scalar.activation(bbox[0:K2, :], psc_b[0:K2, :], Copy, bias=0.0, scale=S)
                if pend is not None:
                    do_finals(c, *pend)
                return (m, abox, bbox, Acur, Bcur)

            # ================= main flow =================
            # stage-1 c0 (p0 chunks loaded first; p1 prefetched during)
            for k in range(8):
                load_p_chunk(0, k)
            for si in range(NS):
                if si < 8:
                    load_p_chunk(1, si)  # prefetch channel 1
                s1_strip(0, si)

            for c in range(3):
                Acur = dict(A_t)
                Bcur = dict(B_t)
                load_ifull(c, 0)
                load_ifull(c, 1)
                if c < 2:
                    ip_strip(c + 1, 0, nc.gpsimd)
                pend = None
                for step in range(NS):
                    pend = ph2_m(c, step, Acur, Bcur, pend)
                    load_ifull(c, step + 2)
                    if c < 2:
                        if step + 1 < NS:
                            ip_strip(c + 1, step + 1, nc.gpsimd)
                        if c == 0 and step < 8:
                            load_p_chunk(2, step)
                        s1_strip(c + 1, step)
                do_finals(c, *pend)

    nc.compile()
    return nc


def kernel(I, p, radius):
    assert int(radius) == R
    I = np.ascontiguousarray(np.asarray(I, np.float32))
    p = np.ascontiguousarray(np.asarray(p, np.float32))
    B = I.shape[0]
    assert I.shape == (B, 1, H, W) and p.shape == (B, 3, H, W)

    if "nc" not in _cache:
        _cache["nc"] = _build()
    nc = _cache["nc"]

    from concourse.bass_utils import run_bass_kernel_spmd

    b7 = _band7_np()
    strips = _strips()
    bm_f = _bandm_np(*strips[0])
    bm_i = _bandm_np(*strips[1])
    bm_l = _bandm_np(*strips[-1])

    in_maps = []
    for i in range(B):
        in_maps.append(
            {
                "I": I[i, 0],
                "p": p[i],
                "band7": b7,
                "bandm_first": bm_f,
                "bandm_int": bm_i,
                "bandm_last": bm_l,
            }
        )
    res = run_bass_kernel_spmd(nc, in_maps, core_ids=list(range(B)))
    out = np.stack([res.results[i]["q"] for i in range(B)], axis=0)
    return out.astype(np.float32)


# revision 16
# speedup vs baseline: 1.0032x; 1.0032x over previous
"""Guided filter (radius=3) on 8x TRN2 NeuronCores, batch-parallel.

Per core: one image. Box filters = banded matmuls on the PE:
  a1: lhsT = image block (stationary), rhs = vertical band -> (w, h') transposed
  a2: lhsT = horizontal band (stationary), rhs = a1 evac    -> (w', h')
Stage-2 (boxes of a, b) repeats the pair, returning to natural layout.

v5: phase-2(c) is interleaved with stage-1(c+1) so the PE-heavy stage-2
windows overlap the Vector/Scalar-heavy stage-1 windows. ab tiles are
double-buffered (bufs=2) so channel c+1's coefficient writes never wait on
channel c's stage-2 reads. I*p product strips are computed one step ahead
on GpSimd (c0 on Vector). ifull is reloaded per channel through a 2-buf
rotation; c0-only tiles share tags with phase-2-only tiles to fit SBUF.
"""

import sys

sys.path.insert(0, "/opt/trn_rl_repo")

import numpy as np
import ml_dtypes

R = 3
H = W = 1024
P = 128
NC_N = 8
V = 122  # valid outputs per 128-wide band matmul
S = float(64.0 / 49.0)

_cache = {}


def _strips():
    # (in_lo, in_hi, out_lo, out_hi) along one axis
    out = []
    j = 0
    while j * V < W:
        o_lo, o_hi = j * V, min(W, j * V + V)
        i_lo, i_hi = max(0, o_lo - R), min(W, o_hi + R)
        out.append((i_lo, i_hi, o_lo, o_hi))
        j += 1
    return out


def _band7_np():
    b = np.zeros((128, 134), np.float32)
    for k in range(128):
        for d in range(134):
            if abs(d - 3 - k) <= R:
                b[k, d] = 0.125
    return b.astype(ml_dtypes.bfloat16)


def _bandm_np(i_lo, i_hi, o_lo, o_hi):
    K = i_hi - i_lo
    bm = np.zeros((K, 128), np.float32)
    for k in range(K):
        for m in range(o_hi - o_lo):
            if abs((i_lo + k) - (o_lo + m)) <= R:
                bm[k, m] = 0.125
    return bm.astype(ml_dtypes.bfloat16)


def _seg512(lo, hi):
    """split [lo,hi) at multiples of 512 (PSUM bank boundaries)"""
    segs = []
    while lo < hi:
        nxt = min(hi, (lo // 512 + 1) * 512)
        segs.append((lo, nxt))
        lo = nxt
    return segs


def _build():
    import concourse.bass as bass
    import concourse.bacc as bacc
    import concourse.mybir as mybir
    from concourse import tile

    bf16 = mybir.dt.bfloat16
    f32 = mybir.dt.float32
    Copy = mybir.ActivationFunctionType.Copy
    Alu = mybir.AluOpType

    strips = _strips()
    NS = len(strips)

    nc = bacc.Bacc(None, target_bir_lowering=False)
    dI = nc.dram_tensor("I", [H, W], f32, kind="ExternalInput")
    dp = nc.dram_tensor("p", [3, H, W], f32, kind="ExternalInput")
    db7 = nc.dram_tensor("band7", [128, 134], bf16, kind="ExternalInput")
    dbm_f = nc.dram_tensor("bandm_first", [125, 128], bf16, kind="ExternalInput")
    dbm_i = nc.dram_tensor("bandm_int", [128, 128], bf16, kind="ExternalInput")
    dbm_l = nc.dram_tensor("bandm_last", [51, 128], bf16, kind="ExternalInput")
    dq = nc.dram_tensor("q", [3, H, W], f32, kind="ExternalOutput")

    with tile.TileContext(nc) as tc:
        with (
            tc.tile_pool(name="const", bufs=1) as constp,
            tc.tile_pool(name="chk", bufs=1) as chp,
            tc.tile_pool(name="perm", bufs=1) as perm,
            tc.tile_pool(name="work", bufs=2) as workp,
            tc.tile_pool(name="ab", bufs=2) as abp,
            tc.tile_pool(name="qo", bufs=2) as qp,
            tc.tile_pool(name="psA", bufs=3, space="PSUM") as psA,
        ):
            band7 = constp.tile([128, 134], bf16, tag="band7")
            nc.sync.dma_start(band7[:], db7.ap()[:])
            bm_first = constp.tile([125, 128], bf16, tag="bmf")
            nc.sync.dma_start(bm_first[:], dbm_f.ap()[:])
            bm_int = constp.tile([128, 128], bf16, tag="bmi")
            nc.sync.dma_start(bm_int[:], dbm_i.ap()[:])
            bm_last = constp.tile([51, 128], bf16, tag="bml")
            nc.sync.dma_start(bm_last[:], dbm_l.ap()[:])

            def bandm_for(si):
                if si == 0:
                    return bm_first
                if si == NS - 1:
                    return bm_last
                return bm_int

            # ---- persistent natural-layout input chunks (gpsimd cast-DMA)
            Ibig = chp.tile([128, 8 * 1024], bf16, tag="Ibig")
            for k in range(8):
                nc.gpsimd.dma_start(
                    Ibig[:, k * 1024 : (k + 1) * 1024], dI.ap()[128 * k : 128 * k + 128, :]
                )
            Ich = [Ibig[:, k * 1024 : (k + 1) * 1024] for k in range(8)]

            pch = {}

            pbig_t = {}

            def load_p_chunk(c, k):
                if c not in pbig_t:
                    pbig_t[c] = chp.tile([128, 8 * 1024], bf16, tag="pbig", bufs=2, name=f"pbig{c}")
                pb = pbig_t[c]
                nc.gpsimd.dma_start(
                    pb[:, k * 1024 : (k + 1) * 1024], dp.ap()[c][128 * k : 128 * k + 128, :]
                )
                pch.setdefault(c, {})[k] = pb[:, k * 1024 : (k + 1) * 1024]

            ipstrip = {}

            def ip_strip(c, si, eng):
                """packed I*p product strip [128, 8*128] (block i at i*128),
                computed as ONE strided tensor_tensor over the big tiles"""
                i_lo, i_hi, _, _ = strips[si]
                Mw = i_hi - i_lo
                t = workp.tile([128, 8 * 128], bf16, tag="ipps", bufs=2)
                tv = t[:].rearrange("p (k j) -> p k j", j=128)[:, :, 0:Mw]
                iv = Ibig[:].rearrange("p (k w) -> p k w", w=1024)[:, :, i_lo:i_hi]
                pv = pbig_t[c][:].rearrange("p (k w) -> p k w", w=1024)[:, :, i_lo:i_hi]
                eng.tensor_mul(tv, iv, pv)
                ipstrip[(c, si)] = t

            def a1_pass(ps, blocks, si, Mw, stride=None):
                """vertical box + transpose: accumulate 8 h-blocks into
                ps[0:Mw, 0:1024]."""
                i_lo, i_hi, _, _ = strips[si]
                seen = set()
                for i in range(8):
                    if stride is None:
                        lhsT = blocks[i][:, i_lo:i_hi]
                    else:
                        lhsT = blocks[:, i * stride : i * stride + Mw]
                    base = 128 * i - 3
                    w_lo_ = max(0, 128 * i - 3)
                    w_hi_ = min(1024, 128 * i + 131)
                    for s_lo, s_hi in _seg512(w_lo_, w_hi_):
                        bank = s_lo // 512
                        nc.tensor.matmul(
                            ps[0:Mw, s_lo:s_hi],
                            lhsT,
                            band7[:, s_lo - base : s_hi - base],
                            start=bank not in seen,
                            stop=True,
                        )
                        seen.add(bank)

            def a2_pass(ps, vt_tile, si, K):
                """horizontal box via band-stationary matmul: ps[0:128, 0:1024]"""
                bm = bandm_for(si)
                for s_lo, s_hi in _seg512(0, 1024):
                    nc.tensor.matmul(
                        ps[:, s_lo:s_hi],
                        bm[:],
                        vt_tile[0:K, s_lo:s_hi],
                        start=True,
                        stop=True,
                    )

            uI_t = {}
            rv_t = {}
            A_t = {}
            B_t = {}

            def b1_pass(ps, tiles, m_lo, m_hi):
                """W-box of a/b over w'-strips + transpose back to natural"""
                seen = set()
                for sj, (ji_lo, ji_hi, jo_lo, jo_hi) in enumerate(strips):
                    Kj = jo_hi - jo_lo
                    t = tiles[sj]
                    lhsT = t[0:Kj, m_lo:m_hi]
                    base = jo_lo - 3
                    w_lo_ = max(0, jo_lo - 3)
                    w_hi_ = min(1024, jo_lo + 125)
                    for s_lo, s_hi in _seg512(w_lo_, w_hi_):
                        bank = s_lo // 512
                        nc.tensor.matmul(
                            ps[0 : m_hi - m_lo, s_lo:s_hi],
                            lhsT,
                            band7[0:Kj, s_lo - base : s_hi - base],
                            start=bank not in seen,
                            stop=True,
                        )
                        seen.add(bank)

            def s1_strip(c, si):
                """stage-1 for one strip: PE a1s then a2s; ACT evacs; DVE math."""
                i_lo, i_hi, o_lo, o_hi = strips[si]
                Mw = i_hi - i_lo
                K_out = o_hi - o_lo
                pc = pch[c]
                if c == 0:
                    # ii strip (packed, DVE) and ip strip (DVE, inline)
                    iis = workp.tile([128, 8 * 128], bf16, tag="iis", bufs=1)
                    iiv = iis[:].rearrange("p (k j) -> p k j", j=128)[:, :, 0:Mw]
                    iv = Ibig[:].rearrange("p (k w) -> p k w", w=1024)[:, :, i_lo:i_hi]
                    nc.vector.tensor_mul(iiv, iv, iv)
                    ip_strip(0, si, nc.vector)

                    psa = psA.tile([128, 1024], f32, tag="mm")
                    a1_pass(psa, Ich, si, Mw)
                    vtI = workp.tile([128, 1024], bf16, tag="vt", bufs=3)
                    nc.scalar.activation(vtI[0:Mw, :], psa[0:Mw, :], Copy, bias=0.0, scale=1.0)

                    psa2 = psA.tile([128, 1024], f32, tag="mm")
                    a1_pass(psa2, iis, si, Mw, stride=128)
                    vtII = workp.tile([128, 1024], bf16, tag="vt", bufs=3)
                    nc.scalar.activation(vtII[0:Mw, :], psa2[0:Mw, :], Copy, bias=0.0, scale=1.0)

                    psa3 = psA.tile([128, 1024], f32, tag="mm")
                    a1_pass(psa3, pc, si, Mw)
                    vtp = workp.tile([128, 1024], bf16, tag="vt", bufs=3)
                    nc.scalar.activation(vtp[0:Mw, :], psa3[0:Mw, :], Copy, bias=0.0, scale=1.0)

                    psa4 = psA.tile([128, 1024], f32, tag="mm")
                    a1_pass(psa4, ipstrip[(c, si)], si, Mw, stride=128)
                    vtip = workp.tile([128, 1024], bf16, tag="vt", bufs=3)
                    nc.scalar.activation(vtip[0:Mw, :], psa4[0:Mw, :], Copy, bias=0.0, scale=1.0)

                    psb = psA.tile([128, 1024], f32, tag="mm")
                    a2_pass(psb, vtI, si, Mw)
                    uI = perm.tile([128, 1024], bf16, tag=f"uI{si}")
                    nc.scalar.activation(uI[0:K_out, :], psb[0:K_out, :], Copy, bias=0.0, scale=S)
                    uI_t[si] = uI

                    psb2 = psA.tile([128, 1024], f32, tag="hold", bufs=1)
                    a2_pass(psb2, vtII, si, Mw)  # held in PSUM; var reads it

                    psb3 = psA.tile([128, 1024], f32, tag="mm")
                    a2_pass(psb3, vtp, si, Mw)
                    up = workp.tile([128, 1024], bf16, tag="up", bufs=2)
                    nc.scalar.activation(up[0:K_out, :], psb3[0:K_out, :], Copy, bias=0.0, scale=S)

                    psb4 = psA.tile([128, 1024], f32, tag="mm")
                    a2_pass(psb4, vtip, si, Mw)
                    uip = workp.tile([128, 1024], bf16, tag="uip", bufs=2)
                    nc.scalar.activation(uip[0:K_out, :], psb4[0:K_out, :], Copy, bias=0.0, scale=S)

                    # var chain: sq (TT 2x, shares t3 slot), var = S*uII - sq, recip
                    sq = workp.tile([128, 1024], bf16, tag="t3", bufs=1)
                    nc.vector.tensor_mul(sq[0:K_out, :], uI[0:K_out, :], uI[0:K_out, :])
                    var = workp.tile([128, 1024], f32, tag="var", bufs=1)
                    nc.vector.scalar_tensor_tensor(
                        var[0:K_out, :], psb2[0:K_out, :], S, sq[0:K_out, :], Alu.mult, Alu.subtract
                    )
                    rvf = workp.tile([128, 1024], f32, tag="rvf", bufs=1)
                    nc.vector.reciprocal_approx_fast(rvf[0:K_out, :], var[0:K_out, :])
                    rv = perm.tile([128, 1024], bf16, tag=f"rv{si}")
                    nc.scalar.activation(rv[0:K_out, :], rvf[0:K_out, :], Copy, bias=0.0, scale=1.0)
                    rv_t[si] = rv
                else:
                    psa3 = psA.tile([128, 1024], f32, tag="mm")
                    a1_pass(psa3, pc, si, Mw)
                    vtp = workp.tile([128, 1024], bf16, tag="vt", bufs=3)
                    nc.scalar.activation(vtp[0:Mw, :], psa3[0:Mw, :], Copy, bias=0.0, scale=1.0)

                    psa4 = psA.tile([128, 1024], f32, tag="mm")
                    a1_pass(psa4, ipstrip[(c, si)], si, Mw, stride=128)
                    vtip = workp.tile([128, 1024], bf16, tag="vt", bufs=3)
                    nc.scalar.activation(vtip[0:Mw, :], psa4[0:Mw, :], Copy, bias=0.0, scale=1.0)

                    psb3 = psA.tile([128, 1024], f32, tag="mm")
                    a2_pass(psb3, vtp, si, Mw)
                    up = workp.tile([128, 1024], bf16, tag="up", bufs=2)
                    nc.scalar.activation(up[0:K_out, :], psb3[0:K_out, :], Copy, bias=0.0, scale=S)

                    psb4 = psA.tile([128, 1024], f32, tag="hold", bufs=1)
                    a2_pass(psb4, vtip, si, Mw)
                    uip = None

                uI = uI_t[si]
                rv = rv_t[si]
                tt = workp.tile([128, 1024], bf16, tag="tt", bufs=1)
                nc.vector.tensor_mul(tt[0:K_out, :], uI[0:K_out, :], up[0:K_out, :])
                cov = workp.tile([128, 1024], bf16, tag="cov", bufs=1)
                if uip is not None:
                    nc.vector.tensor_sub(cov[0:K_out, :], uip[0:K_out, :], tt[0:K_out, :])
                else:
                    nc.vector.scalar_tensor_tensor(
                        cov[0:K_out, :], psb4[0:K_out, :], S, tt[0:K_out, :], Alu.mult, Alu.subtract
                    )
                a_t = abp.tile([128, 1024], bf16, tag=f"a{si}")
                nc.vector.tensor_mul(a_t[0:K_out, :], cov[0:K_out, :], rv[0:K_out, :])
                t3 = workp.tile([128, 1024], bf16, tag="t3", bufs=1)
                nc.vector.tensor_mul(t3[0:K_out, :], a_t[0:K_out, :], uI[0:K_out, :])
                b_t = abp.tile([128, 1024], bf16, tag=f"b{si}")
                nc.vector.tensor_sub(b_t[0:K_out, :], up[0:K_out, :], t3[0:K_out, :])
                A_t[si] = a_t
                B_t[si] = b_t

            ifull = {}

            def load_ifull(c, m):
                if m >= NS:
                    return
                _, _, mo_lo, mo_hi = strips[m]
                ift = perm.tile([128, 1024], bf16, tag="if", bufs=2)
                r = mo_lo
                while r < mo_hi:
                    k = r // 128
                    r_hi = min(mo_hi, 128 * (k + 1))
                    nc.sync.dma_start(
                        ift[r - mo_lo : r_hi - mo_lo, :],
                        Ich[k][r - 128 * k : r_hi - 128 * k, :],
                    )
                    r = r_hi
                ifull[(c, m)] = ift

            def do_finals(c, mm_i, abox_t, bbox_t, Aview, Bview):
                m2i_lo, m2i_hi, m2o_lo, m2o_hi = strips[mm_i]
                Hw = m2o_hi - m2o_lo
                Kf = m2i_hi - m2i_lo
                bm2 = bandm_for(mm_i)
                qbf = qp.tile([128, 1024], bf16, tag="qb", bufs=2)
                for s_lo, s_hi in _seg512(0, 1024):
                    pd = psA.tile([128, 1024], f32, tag="mm")
                    nc.tensor.matmul(
                        pd[:, 0:512], bm2[:], abox_t[0:Kf, s_lo:s_hi], start=True, stop=True
                    )
                    nc.tensor.matmul(
                        pd[:, 512:1024], bm2[:], bbox_t[0:Kf, s_lo:s_hi], start=True, stop=True
                    )
                    tseg = workp.tile([128, 512], bf16, tag="iis", bufs=1)
                    nc.vector.tensor_mul(
                        tseg[0:Hw, :], ifull[(c, mm_i)][0:Hw, s_lo:s_hi], pd[0:Hw, 0:512]
                    )
                    nc.vector.tensor_add(
                        qbf[0:Hw, s_lo:s_hi], tseg[0:Hw, :], pd[0:Hw, 512:1024]
                    )
                nc.vector.tensor_scalar(qbf[0:Hw, :], qbf[0:Hw, :], 1.0, 0.0, Alu.min, Alu.max)
                nc.gpsimd.dma_start(dq.ap()[c][m2o_lo:m2o_hi, :], qbf[0:Hw, :])

            def ph2_m(c, m, Acur, Bcur, pend):
                """phase-2 step: b1 passes + evacs for m, finals for pend"""
                mi_lo, mi_hi, mo_lo, mo_hi = strips[m]
                K2 = mi_hi - mi_lo
                psc_a = psA.tile([128, 1024], f32, tag="mm")
                b1_pass(psc_a, Acur, mi_lo, mi_hi)
                abox = workp.tile([128, 1024], bf16, tag="abox", bufs=2)
                nc.scalar.activation(abox[0:K2, :], psc_a[0:K2, :], Copy, bias=0.0, scale=S)
                psc_b = psA.tile([128, 1024], f32, tag="mm")
                b1_pass(psc_b, Bcur, mi_lo, mi_hi)
                bbox = workp.tile([128, 1024], bf16, tag="bbox", bufs=2)
                nc.scalar.activation(bbox[0:K2, :], psc_b[0:K2, :], Copy, bias=0.0, scale=S)
                if pend is not None:
                    do_finals(c, *pend)
                return (m, abox, bbox, Acur, Bcur)

            # ================= main flow =================
            # stage-1 c0 (p0 chunks loaded first; p1 prefetched during)
            for k in range(8):
                load_p_chunk(0, k)
            for si in range(NS):
                if si < 8:
                    load_p_chunk(1, si)  # prefetch channel 1
                s1_strip(0, si)

            for c in range(3):
                Acur = dict(A_t)
                Bcur = dict(B_t)
                load_ifull(c, 0)
                load_ifull(c, 1)
                if c < 2:
                    ip_strip(c + 1, 0, nc.gpsimd)
                pend = None
                for step in range(NS):
                    pend = ph2_m(c, step, Acur, Bcur, pend)
                    load_ifull(c, step + 2)
                    if c < 2:
                        if step + 1 < NS:
                            ip_strip(c + 1, step + 1, nc.gpsimd)
                        if c == 0 and step < 8:
                            load_p_chunk(2, step)
                        s1_strip(c + 1, step)
                do_finals(c, *pend)

    nc.compile()
    return nc


def kernel(I, p, radius):
    assert int(radius) == R
    I = np.ascontiguousarray(np.asarray(I, np.float32))
    p = np.ascontiguousarray(np.asarray(p, np.float32))
    B = I.shape[0]
    assert I.shape == (B, 1, H, W) and p.shape == (B, 3, H, W)

    if "nc" not in _cache:
        _cache["nc"] = _build()
    nc = _cache["nc"]

    from concourse.bass_utils import run_bass_kernel_spmd

    b7 = _band7_np()
    strips = _strips()
    bm_f = _bandm_np(*strips[0])
    bm_i = _bandm_np(*strips[1])
    bm_l = _bandm_np(*strips[-1])

    in_maps = []
    for i in range(B):
        in_maps.append(
            {
                "I": I[i, 0],
                "p": p[i],
                "band7": b7,
                "bandm_first": bm_f,
                "bandm_int": bm_i,
                "bandm_last": bm_l,
            }
        )
    res = run_bass_kernel_spmd(nc, in_maps, core_ids=list(range(B)))
    out = np.stack([res.results[i]["q"] for i in range(B)], axis=0)
    return out.astype(np.float32)
